# revision 79
# baseline (speedup 1.0000x reference)
"""Trainium2 Bass kernel for a GPT-style transformer block (B=2, T=2048,
C=1024, 16 heads, MLP 4x), sharded across 8 NeuronCores.

Sharding: attention is HEAD-sharded (core = (batch b=c//4, head group
j=c%4, heads 4j..4j+3)); each core computes q/k/v for its 4 heads over
all 2048 tokens of its batch, so no kv exchange is needed at all.
Causal attention runs exact (subchunk s in 0..7 iterates kv tiles
0..2s+1; only the two diagonal tiles get masked, with masks identical
on every core). The AV matmul emits [q-partition, dim] (full 128-lane
PE use) with a ones-column appended to v for the softmax denominator;
normalization is a per-partition scalar multiply, and PE transposes
flip the result to channel-major.

A single 8-way AllToAll (0.5 MB, fp8 payload) re-shards: subchunk s of
every core is dest core s's slice, so core i ends up with tokens
[256i,256i+256) of BOTH batches (512 tokens) for proj + residual + LN2
+ MLP, which run token-sharded exactly like a dense layer stack. The
attn-proj runs as fp8 DoubleRow matmuls (weights pre-scaled x64 on the
host, descaled via the activation's scale); everything feeding softmax
or the MLP stays bf16 (fp8 there was measured to breach the 2e-2
tolerance). proj+LN2+up are pipelined by token half; MLP down-weights
prefetch during attention and up-weights stream just-in-time.

Host precomputes LN1 (inputs-only), folds LN scale/shift and 1/sqrt(D)
into matmul weights, folds the proj bias into the residual, and
pre-transposes everything. Residual stream and softmax normalization
stay fp32; matmuls accumulate in fp32 PSUM.
"""
import numpy as np
import ml_dtypes

import concourse.bass as bass
import concourse.mybir as mybir
import concourse.tile as tile
import concourse.bacc as bacc
from concourse.bass_utils import run_bass_kernel_spmd
from concourse.masks import make_identity

BF = ml_dtypes.bfloat16
P = 128
B, T, C, H, D, F = 2, 2048, 1024, 16, 64, 4096
NCT = C // P          # 8   c-tiles
NFT = F // P          # 32  f-tiles
NKT = T // P          # 16  kv tiles per batch
SUB = 256             # q subchunk rows
EPS = 1e-5
f32 = mybir.dt.float32
bf16 = mybir.dt.bfloat16
fp8 = mybir.dt.float8e4
F8 = mybir.dt.np(fp8)
DR = mybir.MatmulPerfMode.DoubleRow
AF = mybir.ActivationFunctionType
WS = 64.0         # fp8 weight scale (cleared via act scale=1/WS)
IWS = 1.0 / WS

_CACHED_NC = None
DEBUG = False
SKIP_CC = False
PHASE = 5   # 1=qkv 2=+attn 3=+a2a 4=+proj/ln2 5=full
NO_TPS = False   # skip PE transposes (attn_cT left uninitialized garbage)
NO_AV = False    # skip av matmuls + normalize (asb taken from ex)
NO_A2A_DMA = False  # skip the per-subchunk a2a_in DRAM write
NO_EPI = False   # skip the entire per-subchunk epilogue
LOOP_LEVEL = 3   # 0=empty loop 1=sc only 2=+exp 3=full body


def _build_nc():
    nc = bacc.Bacc("TRN2", target_bir_lowering=False, debug=False)
    d = {}
    if DEBUG:
        for name, shape, dt in [
            ("dbg_den", [P, 8], f32), ("dbg_ex", [P, 1024], bf16),
            ("dbg_sc", [P, 1024], f32), ("dbg_qT", [P, 2, T], bf16),
            ("dbg_kT", [P, 2, T], bf16), ("dbg_v4", [P, 4, NKT, 65], bf16),
            ("dbg_acT", [P, 2, T], fp8), ("dbg_attnT", [P, NCT, 512], fp8),
            ("dbg_x1", [P, NCT, 512], f32), ("dbg_g2", [P, NCT, 512], fp8),
        ]:
            d[name] = nc.dram_tensor(name, shape, dt,
                                     kind="ExternalOutput").ap()
    for name, shape, dt in [
        ("gT", [C, T], bf16),
        ("WqT", [C, 256], bf16), ("WkT", [C, 256], bf16),
        ("WvT", [C, 256], bf16),
        ("bq", [P, 2], f32), ("bk", [P, 2], f32), ("brep", [P, 256], bf16),
        ("maskA", [P, 1024], bf16), ("maskB", [P, 1024], bf16),
        ("WpT", [C, C], fp8), ("xbT", [C, 512], f32),
        ("WupT", [NFT, P, NCT, P], bf16), ("bup", [P, NFT], f32),
        ("WdownT", [NFT, P, C], bf16), ("bdown", [P, NCT], f32),
    ]:
        d[name] = nc.dram_tensor(name, shape, dt, kind="ExternalInput").ap()
    d["OUT"] = nc.dram_tensor("OUT", [C, 512], f32, kind="ExternalOutput").ap()

    with tile.TileContext(nc) as tc:
        _emit(nc, tc, d)
    nc.compile()
    return nc


def _emit(nc, tc, d):
    from contextlib import ExitStack

    with ExitStack() as ctx:
        # ---- long-lived pools (creation order = SBUF stack order) ----
        cpool = ctx.enter_context(tc.tile_pool(name="cpool", bufs=1))
        prepool = ctx.enter_context(tc.tile_pool(name="prepool", bufs=1))
        dramp = ctx.enter_context(tc.tile_pool(name="dramp", bufs=1,
                                               space="DRAM"))
        bpool_cm = tc.tile_pool(name="bpool", bufs=1)
        bpool = bpool_cm.__enter__()   # closed explicitly after attention

        # persistent small tiles
        bq = cpool.tile([P, 2], f32, name="bq")
        bk = cpool.tile([P, 2], f32, name="bk")
        brep = cpool.tile([P, 256], bf16, name="brep")
        bup = cpool.tile([P, NFT], f32, name="bup")
        bdown = cpool.tile([P, NCT], f32, name="bdown")
        epsT = cpool.tile([P, 1], f32, name="epsT")
        onesb = cpool.tile([P, P], bf16, name="onesb")
        ident = cpool.tile([P, P], bf16, name="ident")
        maskA = cpool.tile([P, 1024], bf16, name="maskA")
        maskB = cpool.tile([P, 1024], bf16, name="maskB")
        xbT = cpool.tile([P, NCT, 512], f32, name="xbT")
        attnT = cpool.tile([P, NCT, 512], fp8, name="attnT")
        nc.vector.memset(epsT[:], EPS)
        nc.vector.memset(onesb[:], 1.0)
        make_identity(nc, ident[:])
        # small/late-needed loads go on the Pool queue so the sync queue can
        # start streaming gT immediately
        for t, key in [(bq, "bq"), (bk, "bk"), (brep, "brep"), (bup, "bup"),
                       (bdown, "bdown"), (maskA, "maskA"), (maskB, "maskB")]:
            nc.gpsimd.dma_start(t[:], d[key])

        # prefetched weights: proj (used right after a2a) + MLP down
        wpT = prepool.tile([P, NCT, C], fp8, name="wpT")
        wd = prepool.tile([P, NFT, C], bf16, name="wd")

        # attention working set
        qT = bpool.tile([P, 2, T], bf16, name="qT")
        kT = bpool.tile([P, 2, T], bf16, name="kT")
        v4 = bpool.tile([P, 4, NKT, 65], bf16, name="v4")
        attn_cT = bpool.tile([P, 2, T], fp8, name="attn_cT")
        nc.vector.memset(v4[:, :, :, 64:65], 1.0)

        # a2a DRAM staging (fp8 payload: halves the exposed collective)
        a2a_in = dramp.tile([8, 256, 256], fp8, name="a2a_in")
        a2a_out = dramp.tile([8, 256, 256], fp8, name="a2a_out")

        # ================= QKV projections (my 4 heads, all tokens) ========
        with tc.tile_pool(name="qkvp", bufs=1) as qkvp, \
             tc.tile_pool(name="qkps", bufs=4, space="PSUM") as qkps:
            gT = qkvp.tile([P, NCT, T], bf16, name="gT")
            wqT = qkvp.tile([P, NCT, 256], bf16, name="wqT")
            wkT = qkvp.tile([P, NCT, 256], bf16, name="wkT")
            wvT = qkvp.tile([P, NCT, 256], bf16, name="wvT")
            gsrc = d["gT"].rearrange("(ct p) t -> p ct t", p=P)
            nc.sync.dma_start(
                wqT[:], d["WqT"].rearrange("(ct p) o -> p ct o", p=P))
            for tq in range(4):
                for ct in range(NCT):
                    # alternate queues so descriptor issue parallelizes
                    eng = nc.sync if ct % 2 == 0 else nc.gpsimd
                    eng.dma_start(gT[:, ct, tq * 512:(tq + 1) * 512],
                                  gsrc[:, ct, tq * 512:(tq + 1) * 512])
                if tq == 0:
                    nc.sync.dma_start(
                        wkT[:], d["WkT"].rearrange("(ct p) o -> p ct o", p=P))
                elif tq == 1:
                    nc.sync.dma_start(
                        wvT[:], d["WvT"].rearrange("(ct p) o -> p ct o", p=P))
            # q and k: out [128 dims(head pair), tokens]
            for w, bias, dst in ((wqT, bq, qT), (wkT, bk, kT)):
                for tq in range(4):
                    for hp in range(2):
                        pq = qkps.tile([P, 512], f32, name="pq", tag="qk")
                        for ct in range(NCT):
                            nc.tensor.matmul(
                                pq[:], w[:, ct, hp * P:(hp + 1) * P],
                                gT[:, ct, tq * 512:(tq + 1) * 512],
                                start=(ct == 0), stop=(ct == NCT - 1))
                        nc.scalar.add(dst[:, hp, tq * 512:(tq + 1) * 512],
                                      pq[:], bias[:, hp:hp + 1])
            # v: out [128 tokens, 256 dims] -> v4[:, h, kt, 0:64]
            for tt in range(NKT):
                pv = qkps.tile([P, 256], f32, name="pv", tag="qk")
                for ct in range(NCT):
                    nc.tensor.matmul(pv[:], gT[:, ct, tt * P:(tt + 1) * P],
                                     wvT[:, ct, :],
                                     start=(ct == 0), stop=(ct == NCT - 1))
                nc.vector.tensor_add(
                    v4[:, :, tt, 0:64],
                    pv[:].rearrange("p (h e) -> p h e", e=64),
                    brep[:].rearrange("p (h e) -> p h e", e=64))

        if DEBUG:
            nc.sync.dma_start(d["dbg_qT"], qT[:])
            nc.sync.dma_start(d["dbg_kT"], kT[:])
            nc.sync.dma_start(d["dbg_v4"], v4[:])

        if PHASE < 2:
            nc.sync.dma_start(d["OUT"].rearrange("(x p) t -> p x t", p=P),
                              xbT[:])
            bpool_cm.__exit__(None, None, None)
            return

        # prefetch wd + wpT during attention (no deps -> runs immediately)
        nc.sync.dma_start(wpT[:], d["WpT"].rearrange("(ct p) o -> p ct o", p=P))
        nc.sync.dma_start(wd[:], d["WdownT"].rearrange("cf p o -> p cf o"))
        # residual slice: not needed until proj, so keep it off the critical
        # early gT stream and load it during attention instead
        nc.sync.dma_start(xbT[:],
                          d["xbT"].rearrange("(ct p) t -> p ct t", p=P))

        # ================= attention (exact causal) =================
        with tc.tile_pool(name="expp", bufs=7) as expp, \
             tc.tile_pool(name="nrmp", bufs=3) as nrmp, \
             tc.tile_pool(name="asbp", bufs=4) as asbp, \
             tc.tile_pool(name="scps", bufs=2, space="PSUM") as scps, \
             tc.tile_pool(name="avps", bufs=1, space="PSUM") as avps, \
             tc.tile_pool(name="tpsp", bufs=2, space="PSUM") as tpsp:
            # software-pipelined: emit sc(i+1) before exp/av(i) so the PE
            # queue never head-of-line blocks on an av waiting for exp; the
            # epilogue's PE transposes are deferred one stage further so the
            # next sc runs while the DVE normalize chain completes
            items = [(s, kt) for s in range(8) for kt in range(2 * s + 2)]
            avq_s, sc_t = {}, {}

            def alloc_av(s):
                avA = avps.tile([P, 4, 65], f32, name="avA", tag="avA")
                avB = avps.tile([P, 4, 65], f32, name="avB", tag="avB")
                nc.vector.memset(avA[:], 0.0)
                nc.vector.memset(avB[:], 0.0)
                avq_s[s] = (avA, avB)

            def emit_sc(i):
                s, kt = items[i]
                sc = scps.tile([P, 1024], f32, name="sc", tag="sc")
                sc_t[i] = sc
                for h in range(4):
                    hb = (h % 2) * 64
                    colo = (h % 2) * 512 + (h // 2) * 256
                    nc.tensor.matmul(
                        sc[:, colo:colo + 256],
                        kT[hb:hb + 64, h // 2, kt * P:(kt + 1) * P],
                        qT[hb:hb + 64, h // 2, s * SUB:(s + 1) * SUB],
                        start=True, stop=True)

            def emit_rest(i):
                """exp/mask/av; at subchunk end also the DVE normalize.
                Returns (s, asb) at a subchunk boundary, else None."""
                s, kt = items[i]
                nkv = 2 * s + 2
                avq = avq_s[s]
                ex = expp.tile([P, 1024], bf16, name="ex", tag="ex")
                nc.scalar.activation(ex[:], sc_t.pop(i)[:], AF.Exp)
                if kt == 2 * s:
                    nc.vector.tensor_mul(ex[:], ex[:], maskA[:])
                elif kt == 2 * s + 1:
                    nc.vector.tensor_mul(ex[:], ex[:], maskB[:])
                for qt in range(2):
                    for h in range(4):
                        colo = (h % 2) * 512 + (h // 2) * 256
                        nc.tensor.matmul(
                            avq[qt][:, h, :],
                            ex[:, colo + qt * P:colo + (qt + 1) * P],
                            v4[:, h, kt, :],
                            start=False, stop=(kt == nkv - 1),
                            skip_group_check=True)
                if kt != nkv - 1:
                    return None
                avq = avq_s.pop(s)
                den = nrmp.tile([P, 8], f32, name="den", tag="den")
                rden = nrmp.tile([P, 8], f32, name="rden", tag="rden")
                for qt in range(2):
                    nc.vector.tensor_copy(
                        den[:, qt * 4:(qt + 1) * 4],
                        avq[qt][:, :, 64:65].rearrange("p h e -> p (h e)"))
                nc.vector.reciprocal(rden[:], den[:])
                asb = asbp.tile([P, 2, 256], bf16, name="asb", tag="asb")
                for qt in range(2):
                    for h in range(4):
                        nc.vector.tensor_scalar_mul(
                            asb[:, qt, h * 64:(h + 1) * 64],
                            avq[qt][:, h, 0:64],
                            rden[:, qt * 4 + h:qt * 4 + h + 1])
                if s + 1 < 8:
                    alloc_av(s + 1)   # after the normalize reads (bufs=1)
                return (s, asb)

            def emit_tps(s, asb):
                for qt in range(2):
                    for dt in range(2):
                        tps = tpsp.tile([P, P], bf16, name="tps", tag="tps")
                        nc.tensor.transpose(tps[:],
                                            asb[:, qt, dt * P:(dt + 1) * P],
                                            ident[:])
                        nc.vector.tensor_copy(
                            attn_cT[:, dt, s * SUB + qt * P:
                                    s * SUB + (qt + 1) * P],
                            tps[:])
                nc.gpsimd.dma_start(
                    a2a_in[s].rearrange("(dt p) t -> p dt t", p=P),
                    attn_cT[:, :, s * SUB:(s + 1) * SUB])

            alloc_av(0)
            pend = None
            for i in range(len(items) + 2):
                if i < len(items):
                    emit_sc(i)
                if pend is not None:
                    emit_tps(*pend)
                    pend = None
                if 1 <= i <= len(items):
                    pend = emit_rest(i - 1)

        if DEBUG:
            nc.sync.dma_start(d["dbg_acT"], attn_cT[:])

        bpool_cm.__exit__(None, None, None)   # free qT/kT/v4/attn_cT

        if PHASE < 3:
            nc.sync.dma_start(d["OUT"].rearrange("(x p) t -> p x t", p=P),
                              xbT[:])
            return

        # ================= AllToAll: re-shard heads -> tokens ==============
        if SKIP_CC:
            nc.gpsimd.dma_start(a2a_out[:], a2a_in[:])
        else:
            nc.gpsimd.collective_compute(
                "AllToAll", mybir.AluOpType.bypass,
                ins=[a2a_in[:]], outs=[a2a_out[:]],
                replica_groups=[[0, 1, 2, 3, 4, 5, 6, 7]])
        for half in range(2):
            for j in range(4):
                nc.gpsimd.dma_start(
                    attnT[:, 2 * j:2 * j + 2, half * 256:(half + 1) * 256],
                    a2a_out[4 * half + j].rearrange("(dt p) t -> p dt t", p=P))

        if DEBUG:
            nc.sync.dma_start(d["dbg_attnT"], attnT[:])

        if PHASE < 4:
            nc.sync.dma_start(d["OUT"].rearrange("(x p) t -> p x t", p=P),
                              xbT[:])
            return

        # ================= MLP up-weight stream (just-in-time) =============
        with tc.tile_pool(name="mlpp", bufs=1) as mlpp, \
             tc.tile_pool(name="wup", bufs=6) as wup:
            wu_tiles = []
            for ot in range(NFT):
                wt = wup.tile([P, NCT, P], bf16, name="wu", tag="wu")
                nc.sync.dma_start(wt[:], d["WupT"][ot])
                wu_tiles.append(wt)

            x1T = mlpp.tile([P, NCT, 512], f32, name="x1T")
            g2 = mlpp.tile([P, NCT, 512], bf16, name="g2")
            hT = mlpp.tile([P, NFT, 512], bf16, name="hT")

            # ---- proj + residual + LN2 + MLP up, pipelined by token half ----
            # emission order keeps PE fed: proj(0), proj(1), ones(0), ones(1);
            # LN2 stats/normalize for half 0 run on ACT/DVE while PE starts
            # up(0); half 1's LN2 hides under up(0)'s matmuls.
            with tc.tile_pool(name="ln2p", bufs=1) as ln2p, \
                 tc.tile_pool(name="pps", bufs=2, space="PSUM") as pps, \
                 tc.tile_pool(name="lnps", bufs=4, space="PSUM") as lnps, \
                 tc.tile_pool(name="upps", bufs=2, space="PSUM") as upps:
                x1b = ln2p.tile([P, NCT, 512], bf16, name="x1b")
                sqb = ln2p.tile([P, NCT, 512], bf16, name="sqb")
                stats = {}

                def emit_proj(th):
                    sl = slice(th * 256, (th + 1) * 256)
                    for ot in range(NCT):
                        pp = pps.tile([P, 256], f32, name="pp", tag="pp")
                        for dk in range(4):
                            nc.tensor.matmul(
                                pp[:], wpT[:, 2 * dk:2 * dk + 2,
                                           ot * P:(ot + 1) * P],
                                attnT[:, 2 * dk:2 * dk + 2, sl],
                                start=(dk == 0), stop=(dk == 3), perf_mode=DR)
                        ps = ln2p.tile([P, 256], f32, name="ps", tag="ps",
                                       bufs=2)
                        nc.scalar.mul(ps[:], pp[:], IWS)
                        nc.vector.tensor_add(x1T[:, ot, sl], ps[:],
                                             xbT[:, ot, sl])
                        nc.vector.tensor_copy(x1b[:, ot, sl], x1T[:, ot, sl])
                        nc.scalar.activation(sqb[:, ot, sl], x1T[:, ot, sl],
                                             AF.Square)

                def emit_ones(th):
                    sl = slice(th * 256, (th + 1) * 256)
                    psmu = lnps.tile([P, 256], f32, name="psmu", tag="ln")
                    pssq = lnps.tile([P, 256], f32, name="pssq", tag="ln")
                    for ct in range(NCT):
                        nc.tensor.matmul(psmu[:], onesb[:], x1b[:, ct, sl],
                                         start=(ct == 0), stop=(ct == NCT - 1))
                    for ct in range(NCT):
                        nc.tensor.matmul(pssq[:], onesb[:], sqb[:, ct, sl],
                                         start=(ct == 0), stop=(ct == NCT - 1))
                    stats[th] = (psmu, pssq)

                def emit_norm(th):
                    sl = slice(th * 256, (th + 1) * 256)
                    psmu, pssq = stats.pop(th)
                    mu = ln2p.tile([P, 256], f32, name="mu", tag="mu", bufs=2)
                    nc.scalar.mul(mu[:], psmu[:], 1.0 / C)
                    e2 = ln2p.tile([P, 256], f32, name="e2", tag="e2", bufs=2)
                    nc.scalar.mul(e2[:], pssq[:], 1.0 / C)
                    musq = ln2p.tile([P, 256], f32, name="musq", tag="musq",
                                     bufs=2)
                    nc.scalar.activation(musq[:], mu[:], AF.Square)
                    nc.vector.tensor_sub(e2[:], e2[:], musq[:])
                    std = ln2p.tile([P, 256], f32, name="std", tag="std",
                                    bufs=2)
                    nc.scalar.activation(std[:], e2[:], AF.Sqrt, bias=epsT[:])
                    nc.vector.reciprocal(std[:], std[:])
                    for ct in range(NCT):
                        tmpc = ln2p.tile([P, 256], f32, name="tmpc",
                                         tag="tmpc", bufs=2)
                        nc.vector.tensor_sub(tmpc[:], x1T[:, ct, sl], mu[:])
                        nc.vector.tensor_mul(g2[:, ct, sl], tmpc[:], std[:])

                def emit_up():
                    # both halves per f-tile so each wu stream buffer is
                    # fully consumed before its reuse (avoids WAR deadlock)
                    for ot in range(NFT):
                        for th in range(2):
                            sl = slice(th * 256, (th + 1) * 256)
                            pu = upps.tile([P, 256], f32, name="pu", tag="pu")
                            for ct in range(NCT):
                                nc.tensor.matmul(pu[:], wu_tiles[ot][:, ct, :],
                                                 g2[:, ct, sl],
                                                 start=(ct == 0),
                                                 stop=(ct == NCT - 1))
                            nc.scalar.activation(hT[:, ot, sl], pu[:], AF.Gelu,
                                                 bias=bup[:, ot:ot + 1])

                emit_proj(0)
                emit_proj(1)
                emit_ones(0)
                emit_ones(1)
                emit_norm(0)
                emit_norm(1)
                emit_up()

            if DEBUG:
                nc.sync.dma_start(d["dbg_x1"], x1T[:])
                nc.sync.dma_start(d["dbg_g2"], g2[:])

            # ---- MLP down + residual ----
            with tc.tile_pool(name="outp", bufs=1) as outp, \
                 tc.tile_pool(name="dps", bufs=2, space="PSUM") as dps:
                outdst = d["OUT"].rearrange("(ot p) t -> ot p t", p=P)
                for ot in range(NCT):
                    pd = dps.tile([P, 512], f32, name="pd", tag="pd")
                    for cf in range(NFT):
                        nc.tensor.matmul(pd[:], wd[:, cf, ot * P:(ot + 1) * P],
                                         hT[:, cf, :],
                                         start=(cf == 0), stop=(cf == NFT - 1))
                    td = outp.tile([P, 512], f32, name="td", tag="td", bufs=2)
                    nc.scalar.add(td[:], pd[:], bdown[:, ot:ot + 1])
                    ou = outp.tile([P, 512], f32, name="ou", tag="ou", bufs=2)
                    nc.vector.tensor_add(ou[:], td[:], x1T[:, ot, :])
                    nc.gpsimd.dma_start(outdst[ot], ou[:])


def _prep_inputs(x, ln1_w, ln1_b, c_attn_w, c_attn_b, c_proj_w, c_proj_b,
                 ln2_w, ln2_b, up_w, up_b, down_w, down_b):
    """Host-side preprocessing -> list of 8 per-core input dicts."""
    x = np.asarray(x, np.float32)
    f64 = np.float64
    # LN1 on host (pure function of the input)
    mu = x.mean(-1, keepdims=True, dtype=f64)
    var = np.asarray(x, f64).var(-1, keepdims=True)
    g = ((x - mu) / np.sqrt(var + EPS)).astype(np.float32)     # [B, T, C]

    ln1_w = np.asarray(ln1_w, np.float32); ln1_b = np.asarray(ln1_b, np.float32)
    ln2_w = np.asarray(ln2_w, np.float32); ln2_b = np.asarray(ln2_b, np.float32)
    c_attn_w = np.asarray(c_attn_w, np.float32)
    c_attn_b = np.asarray(c_attn_b, np.float32)
    c_proj_w = np.asarray(c_proj_w, np.float32)
    c_proj_b = np.asarray(c_proj_b, np.float32)
    up_w = np.asarray(up_w, np.float32); up_b = np.asarray(up_b, np.float32)
    down_w = np.asarray(down_w, np.float32)
    down_b = np.asarray(down_b, np.float32)

    Wa = c_attn_w * ln1_w[None, :]                  # fold LN1 scale
    ba = c_attn_b + c_attn_w @ ln1_b                # fold LN1 shift
    Wq, Wk, Wv = Wa[:C], Wa[C:2 * C], Wa[2 * C:]
    bqv, bkv, bvv = ba[:C], ba[C:2 * C], ba[2 * C:]
    s = 1.0 / np.sqrt(D)
    Wq = Wq * s; bqv = bqv * s                      # fold attention scale

    Wup = up_w * ln2_w[None, :]
    bupv = up_b + up_w @ ln2_b

    # causal masks for kv tile vs 256-row q subchunk (diagonal tiles), same
    # on every core; tiled x4 across the 4 packed heads
    tk = np.arange(P)[:, None]
    tq = np.arange(SUB)[None, :]
    mA = np.tile((tk <= tq).astype(np.float32), (1, 4))
    mB = np.tile((tk + P <= tq).astype(np.float32), (1, 4))

    shared = {
        "WpT": np.ascontiguousarray(c_proj_w.T * WS).astype(F8),
        "WupT": np.ascontiguousarray(
            Wup.reshape(NFT, P, NCT, P).transpose(0, 3, 2, 1)).astype(BF),
        "bup": np.ascontiguousarray(bupv.reshape(NFT, P).T).astype(np.float32),
        "WdownT": np.ascontiguousarray(
            down_w.T.reshape(NFT, P, C)).astype(BF),
        "bdown": np.ascontiguousarray(
            down_b.reshape(NCT, P).T).astype(np.float32),
        "maskA": mA.astype(BF), "maskB": mB.astype(BF),
    }

    xb = x + c_proj_b[None, None, :]                # fold proj bias in residual
    gT_b = [np.ascontiguousarray(g[b].T).astype(BF) for b in range(B)]
    xbT_b = [np.ascontiguousarray(xb[b].T).astype(np.float32) for b in range(B)]

    # head-pair reorder for q/k: col (hp*128 + (h%2)*64 + d) = head 4j+2hp+(h%2)
    def qk_slice(W, bias, j):
        rows = W.reshape(H, D, C)[4 * j:4 * j + 4]      # [4, 64, C]
        b4 = bias.reshape(H, D)[4 * j:4 * j + 4]
        order = [0, 1, 2, 3]                            # hp0: h0,h1; hp1: h2,h3
        rows = rows[order].reshape(2, 2 * D, C)         # [hp, 128, C]
        b4 = b4[order].reshape(2, 2 * D)
        WT = np.ascontiguousarray(rows.reshape(256, C).T).astype(BF)
        bT = np.ascontiguousarray(b4.reshape(2, P).T).astype(np.float32)
        return WT, bT

    in_maps = []
    for core in range(8):
        b, j = core // 4, core % 4
        m = dict(shared)
        m["gT"] = gT_b[b]
        WqTs, bqs = qk_slice(Wq, bqv, j)
        WkTs, bks = qk_slice(Wk, bkv, j)
        m["WqT"] = WqTs; m["bq"] = bqs
        m["WkT"] = WkTs; m["bk"] = bks
        m["WvT"] = np.ascontiguousarray(
            Wv[256 * j:256 * (j + 1)].T).astype(BF)
        m["brep"] = np.broadcast_to(
            bvv[256 * j:256 * (j + 1)].astype(BF), (P, 256)).copy()
        m["xbT"] = np.concatenate(
            [xbT_b[0][:, 256 * core:256 * (core + 1)],
             xbT_b[1][:, 256 * core:256 * (core + 1)]], axis=1)
        in_maps.append(m)
    return in_maps


def kernel(**inputs):
    global _CACHED_NC
    if _CACHED_NC is None:
        _CACHED_NC = _build_nc()
    nc = _CACHED_NC
    in_maps = _prep_inputs(**inputs)
    try:
        res = run_bass_kernel_spmd(nc, in_maps, list(range(8)))
    except Exception:
        # one retry: transient NRT device faults are recoverable on re-run
        res = run_bass_kernel_spmd(nc, in_maps, list(range(8)))
    out = np.empty((B, T, C), np.float32)
    for core in range(8):
        o = res.results[core]["OUT"]                # [C, 512]
        out[0, 256 * core:256 * (core + 1), :] = o[:, :256].T
        out[1, 256 * core:256 * (core + 1), :] = o[:, 256:].T
    return out


# revision 80
# speedup vs baseline: 1.1262x; 1.1262x over previous
"""Trainium2 Bass kernel for a GPT-style transformer block (B=2, T=2048,
C=1024, 16 heads, MLP 4x), sharded across 8 NeuronCores.

Sharding: attention is HEAD-sharded (core = (batch b=c//4, head group
j=c%4, heads 4j..4j+3)); each core computes q/k/v for its 4 heads over
all 2048 tokens of its batch, so no kv exchange is needed at all.
Causal attention runs exact (subchunk s in 0..7 iterates kv tiles
0..2s+1; only the two diagonal tiles get masked, with masks identical
on every core). The AV matmul emits [q-partition, dim] (full 128-lane
PE use) with a ones-column appended to v for the softmax denominator;
normalization is a per-partition scalar multiply, and PE transposes
flip the result to channel-major.

A single 8-way AllToAll (0.5 MB, fp8 payload) re-shards: subchunk s of
every core is dest core s's slice, so core i ends up with tokens
[256i,256i+256) of BOTH batches (512 tokens) for proj + residual + LN2
+ MLP, which run token-sharded exactly like a dense layer stack. The
attn-proj runs as fp8 DoubleRow matmuls (weights pre-scaled x64 on the
host, descaled via the activation's scale); everything feeding softmax
or the MLP stays bf16 (fp8 there was measured to breach the 2e-2
tolerance). proj+LN2+up are pipelined by token half; MLP down-weights
prefetch during attention and up-weights stream just-in-time.

Host precomputes LN1 (inputs-only), folds LN scale/shift and 1/sqrt(D)
into matmul weights, folds the proj bias into the residual, and
pre-transposes everything. Residual stream and softmax normalization
stay fp32; matmuls accumulate in fp32 PSUM.
"""
import numpy as np
import ml_dtypes

import concourse.bass as bass
import concourse.mybir as mybir
import concourse.tile as tile
import concourse.bacc as bacc
from concourse.bass_utils import run_bass_kernel_spmd
from concourse.masks import make_identity

BF = ml_dtypes.bfloat16
P = 128
B, T, C, H, D, F = 2, 2048, 1024, 16, 64, 4096
NCT = C // P          # 8   c-tiles
NFT = F // P          # 32  f-tiles
NKT = T // P          # 16  kv tiles per batch
SUB = 256             # q subchunk rows
EPS = 1e-5
f32 = mybir.dt.float32
bf16 = mybir.dt.bfloat16
fp8 = mybir.dt.float8e4
F8 = mybir.dt.np(fp8)
DR = mybir.MatmulPerfMode.DoubleRow
AF = mybir.ActivationFunctionType
WS = 64.0         # fp8 weight scale (cleared via act scale=1/WS)
IWS = 1.0 / WS

_CACHED_NC = None
DEBUG = False
SKIP_CC = False
PHASE = 5   # 1=qkv 2=+attn 3=+a2a 4=+proj/ln2 5=full
NO_TPS = False   # skip PE transposes (attn_cT left uninitialized garbage)
NO_AV = False    # skip av matmuls + normalize (asb taken from ex)
NO_A2A_DMA = False  # skip the per-subchunk a2a_in DRAM write
NO_EPI = False   # skip the entire per-subchunk epilogue
LOOP_LEVEL = 3   # 0=empty loop 1=sc only 2=+exp 3=full body


def _build_nc():
    nc = bacc.Bacc("TRN2", target_bir_lowering=False, debug=False)
    d = {}
    if DEBUG:
        for name, shape, dt in [
            ("dbg_den", [P, 8], f32), ("dbg_ex", [P, 1024], bf16),
            ("dbg_sc", [P, 1024], f32), ("dbg_qT", [P, 2, T], bf16),
            ("dbg_kT", [P, 2, T], bf16), ("dbg_v4", [P, 4, NKT, 65], bf16),
            ("dbg_acT", [P, 2, T], fp8), ("dbg_attnT", [P, NCT, 512], fp8),
            ("dbg_x1", [P, NCT, 512], f32), ("dbg_g2", [P, NCT, 512], fp8),
        ]:
            d[name] = nc.dram_tensor(name, shape, dt,
                                     kind="ExternalOutput").ap()
    for name, shape, dt in [
        ("gT", [C, T], bf16),
        ("WqT", [C, 256], bf16), ("WkT", [C, 256], bf16),
        ("WvT", [C, 256], bf16),
        ("bq", [P, 2], f32), ("bk", [P, 2], f32), ("brep", [P, 256], bf16),
        ("maskA", [P, 1024], bf16), ("maskB", [P, 1024], bf16),
        ("WpT", [C, C], fp8), ("xbT", [C, 512], f32),
        ("WupT", [NFT, P, NCT, P], bf16), ("bup", [P, NFT], f32),
        ("WdownT", [NFT, P, C], bf16), ("bdown", [P, NCT], f32),
    ]:
        d[name] = nc.dram_tensor(name, shape, dt, kind="ExternalInput").ap()
    d["OUT"] = nc.dram_tensor("OUT", [C, 512], f32, kind="ExternalOutput").ap()

    with tile.TileContext(nc) as tc:
        _emit(nc, tc, d)
    nc.compile()
    return nc


def _emit(nc, tc, d):
    from contextlib import ExitStack

    with ExitStack() as ctx:
        # ---- long-lived pools (creation order = SBUF stack order) ----
        cpool = ctx.enter_context(tc.tile_pool(name="cpool", bufs=1))
        prepool = ctx.enter_context(tc.tile_pool(name="prepool", bufs=1))
        dramp = ctx.enter_context(tc.tile_pool(name="dramp", bufs=1,
                                               space="DRAM"))
        bpool_cm = tc.tile_pool(name="bpool", bufs=1)
        bpool = bpool_cm.__enter__()   # closed explicitly after attention

        # persistent small tiles
        bq = cpool.tile([P, 2], f32, name="bq")
        bk = cpool.tile([P, 2], f32, name="bk")
        brep = cpool.tile([P, 256], bf16, name="brep")
        bup = cpool.tile([P, NFT], f32, name="bup")
        bdown = cpool.tile([P, NCT], f32, name="bdown")
        epsT = cpool.tile([P, 1], f32, name="epsT")
        onesb = cpool.tile([P, P], bf16, name="onesb")
        ident = cpool.tile([P, P], bf16, name="ident")
        maskA = cpool.tile([P, 1024], bf16, name="maskA")
        maskB = cpool.tile([P, 1024], bf16, name="maskB")
        xbT = cpool.tile([P, NCT, 512], f32, name="xbT")
        attnT = cpool.tile([P, NCT, 512], fp8, name="attnT")
        nc.vector.memset(epsT[:], EPS)
        nc.vector.memset(onesb[:], 1.0)
        make_identity(nc, ident[:])
        # small/late-needed loads go on the Pool queue so the sync queue can
        # start streaming gT immediately
        for t, key in [(bq, "bq"), (bk, "bk"), (brep, "brep"), (bup, "bup"),
                       (bdown, "bdown"), (maskA, "maskA"), (maskB, "maskB")]:
            nc.gpsimd.dma_start(t[:], d[key])

        # prefetched weights: proj (used right after a2a) + MLP down
        wpT = prepool.tile([P, NCT, C], fp8, name="wpT")
        wd = prepool.tile([P, NFT, C], bf16, name="wd")

        # attention working set
        qT = bpool.tile([P, 2, T], bf16, name="qT")
        kT = bpool.tile([P, 2, T], bf16, name="kT")
        v4 = bpool.tile([P, 4, NKT, 65], bf16, name="v4")
        attn_cT = bpool.tile([P, 2, T], fp8, name="attn_cT")
        nc.vector.memset(v4[:, :, :, 64:65], 1.0)

        # a2a DRAM staging (fp8 payload: halves the exposed collective)
        a2a_in = dramp.tile([8, 256, 256], fp8, name="a2a_in")
        a2a_out = dramp.tile([8, 256, 256], fp8, name="a2a_out")

        # ================= QKV projections (my 4 heads, all tokens) ========
        with tc.tile_pool(name="qkvp", bufs=1) as qkvp, \
             tc.tile_pool(name="qkps", bufs=4, space="PSUM") as qkps:
            gT = qkvp.tile([P, NCT, T], bf16, name="gT")
            wqT = qkvp.tile([P, NCT, 256], bf16, name="wqT")
            wkT = qkvp.tile([P, NCT, 256], bf16, name="wkT")
            wvT = qkvp.tile([P, NCT, 256], bf16, name="wvT")
            gsrc = d["gT"].rearrange("(ct p) t -> p ct t", p=P)
            nc.sync.dma_start(
                wqT[:], d["WqT"].rearrange("(ct p) o -> p ct o", p=P))
            for tq in range(4):
                for ct in range(NCT):
                    nc.sync.dma_start(gT[:, ct, tq * 512:(tq + 1) * 512],
                                      gsrc[:, ct, tq * 512:(tq + 1) * 512])
                if tq == 0:
                    nc.sync.dma_start(
                        wkT[:], d["WkT"].rearrange("(ct p) o -> p ct o", p=P))
                elif tq == 1:
                    nc.sync.dma_start(
                        wvT[:], d["WvT"].rearrange("(ct p) o -> p ct o", p=P))
            # q and k: out [128 dims(head pair), tokens]
            for w, bias, dst in ((wqT, bq, qT), (wkT, bk, kT)):
                for tq in range(4):
                    for hp in range(2):
                        pq = qkps.tile([P, 512], f32, name="pq", tag="qk")
                        for ct in range(NCT):
                            nc.tensor.matmul(
                                pq[:], w[:, ct, hp * P:(hp + 1) * P],
                                gT[:, ct, tq * 512:(tq + 1) * 512],
                                start=(ct == 0), stop=(ct == NCT - 1))
                        nc.scalar.add(dst[:, hp, tq * 512:(tq + 1) * 512],
                                      pq[:], bias[:, hp:hp + 1])
            # v: out [128 tokens, 256 dims] -> v4[:, h, kt, 0:64]
            for tt in range(NKT):
                pv = qkps.tile([P, 256], f32, name="pv", tag="qk")
                for ct in range(NCT):
                    nc.tensor.matmul(pv[:], gT[:, ct, tt * P:(tt + 1) * P],
                                     wvT[:, ct, :],
                                     start=(ct == 0), stop=(ct == NCT - 1))
                nc.vector.tensor_add(
                    v4[:, :, tt, 0:64],
                    pv[:].rearrange("p (h e) -> p h e", e=64),
                    brep[:].rearrange("p (h e) -> p h e", e=64))

        if DEBUG:
            nc.sync.dma_start(d["dbg_qT"], qT[:])
            nc.sync.dma_start(d["dbg_kT"], kT[:])
            nc.sync.dma_start(d["dbg_v4"], v4[:])

        if PHASE < 2:
            nc.sync.dma_start(d["OUT"].rearrange("(x p) t -> p x t", p=P),
                              xbT[:])
            bpool_cm.__exit__(None, None, None)
            return

        # prefetch wd + wpT during attention (no deps -> runs immediately)
        nc.sync.dma_start(wpT[:], d["WpT"].rearrange("(ct p) o -> p ct o", p=P))
        nc.sync.dma_start(wd[:], d["WdownT"].rearrange("cf p o -> p cf o"))
        # residual slice: not needed until proj, so keep it off the critical
        # early gT stream and load it during attention instead
        nc.sync.dma_start(xbT[:],
                          d["xbT"].rearrange("(ct p) t -> p ct t", p=P))

        # ================= attention (exact causal) =================
        with tc.tile_pool(name="expp", bufs=7) as expp, \
             tc.tile_pool(name="nrmp", bufs=3) as nrmp, \
             tc.tile_pool(name="asbp", bufs=4) as asbp, \
             tc.tile_pool(name="scps", bufs=2, space="PSUM") as scps, \
             tc.tile_pool(name="avps", bufs=1, space="PSUM") as avps, \
             tc.tile_pool(name="tpsp", bufs=2, space="PSUM") as tpsp:
            # software-pipelined: emit sc(i+1) before exp/av(i) so the PE
            # queue never head-of-line blocks on an av waiting for exp; the
            # epilogue's PE transposes are deferred one stage further so the
            # next sc runs while the DVE normalize chain completes
            items = [(s, kt) for s in range(8) for kt in range(2 * s + 2)]
            avq_s, sc_t = {}, {}

            def alloc_av(s):
                avA = avps.tile([P, 4, 65], f32, name="avA", tag="avA")
                avB = avps.tile([P, 4, 65], f32, name="avB", tag="avB")
                nc.vector.memset(avA[:], 0.0)
                nc.vector.memset(avB[:], 0.0)
                avq_s[s] = (avA, avB)

            def emit_sc(i):
                s, kt = items[i]
                sc = scps.tile([P, 1024], f32, name="sc", tag="sc")
                sc_t[i] = sc
                for h in range(4):
                    hb = (h % 2) * 64
                    colo = (h % 2) * 512 + (h // 2) * 256
                    nc.tensor.matmul(
                        sc[:, colo:colo + 256],
                        kT[hb:hb + 64, h // 2, kt * P:(kt + 1) * P],
                        qT[hb:hb + 64, h // 2, s * SUB:(s + 1) * SUB],
                        start=True, stop=True)

            def emit_rest(i):
                """exp/mask/av; at subchunk end also the DVE normalize.
                Returns (s, asb) at a subchunk boundary, else None."""
                s, kt = items[i]
                nkv = 2 * s + 2
                avq = avq_s[s]
                ex = expp.tile([P, 1024], bf16, name="ex", tag="ex")
                nc.scalar.activation(ex[:], sc_t.pop(i)[:], AF.Exp)
                if kt == 2 * s:
                    nc.vector.tensor_mul(ex[:], ex[:], maskA[:])
                elif kt == 2 * s + 1:
                    nc.vector.tensor_mul(ex[:], ex[:], maskB[:])
                for qt in range(2):
                    for h in range(4):
                        colo = (h % 2) * 512 + (h // 2) * 256
                        nc.tensor.matmul(
                            avq[qt][:, h, :],
                            ex[:, colo + qt * P:colo + (qt + 1) * P],
                            v4[:, h, kt, :],
                            start=False, stop=(kt == nkv - 1),
                            skip_group_check=True)
                if kt != nkv - 1:
                    return None
                avq = avq_s.pop(s)
                den = nrmp.tile([P, 8], f32, name="den", tag="den")
                rden = nrmp.tile([P, 8], f32, name="rden", tag="rden")
                for qt in range(2):
                    nc.vector.tensor_copy(
                        den[:, qt * 4:(qt + 1) * 4],
                        avq[qt][:, :, 64:65].rearrange("p h e -> p (h e)"))
                nc.vector.reciprocal(rden[:], den[:])
                asb = asbp.tile([P, 2, 256], bf16, name="asb", tag="asb")
                for qt in range(2):
                    for h in range(4):
                        nc.vector.tensor_scalar_mul(
                            asb[:, qt, h * 64:(h + 1) * 64],
                            avq[qt][:, h, 0:64],
                            rden[:, qt * 4 + h:qt * 4 + h + 1])
                if s + 1 < 8:
                    alloc_av(s + 1)   # after the normalize reads (bufs=1)
                return (s, asb)

            def emit_tps(s, asb):
                for qt in range(2):
                    for dt in range(2):
                        tps = tpsp.tile([P, P], bf16, name="tps", tag="tps")
                        nc.tensor.transpose(tps[:],
                                            asb[:, qt, dt * P:(dt + 1) * P],
                                            ident[:])
                        nc.vector.tensor_copy(
                            attn_cT[:, dt, s * SUB + qt * P:
                                    s * SUB + (qt + 1) * P],
                            tps[:])
                nc.gpsimd.dma_start(
                    a2a_in[s].rearrange("(dt p) t -> p dt t", p=P),
                    attn_cT[:, :, s * SUB:(s + 1) * SUB])

            alloc_av(0)
            pend = None
            for i in range(len(items) + 2):
                if i < len(items):
                    emit_sc(i)
                if pend is not None:
                    emit_tps(*pend)
                    pend = None
                if 1 <= i <= len(items):
                    pend = emit_rest(i - 1)

        if DEBUG:
            nc.sync.dma_start(d["dbg_acT"], attn_cT[:])

        bpool_cm.__exit__(None, None, None)   # free qT/kT/v4/attn_cT

        if PHASE < 3:
            nc.sync.dma_start(d["OUT"].rearrange("(x p) t -> p x t", p=P),
                              xbT[:])
            return

        # ================= AllToAll: re-shard heads -> tokens ==============
        if SKIP_CC:
            nc.gpsimd.dma_start(a2a_out[:], a2a_in[:])
        else:
            nc.gpsimd.collective_compute(
                "AllToAll", mybir.AluOpType.bypass,
                ins=[a2a_in[:]], outs=[a2a_out[:]],
                replica_groups=[[0, 1, 2, 3, 4, 5, 6, 7]])
        for half in range(2):
            for j in range(4):
                nc.gpsimd.dma_start(
                    attnT[:, 2 * j:2 * j + 2, half * 256:(half + 1) * 256],
                    a2a_out[4 * half + j].rearrange("(dt p) t -> p dt t", p=P))

        if DEBUG:
            nc.sync.dma_start(d["dbg_attnT"], attnT[:])

        if PHASE < 4:
            nc.sync.dma_start(d["OUT"].rearrange("(x p) t -> p x t", p=P),
                              xbT[:])
            return

        # ================= MLP up-weight stream (just-in-time) =============
        with tc.tile_pool(name="mlpp", bufs=1) as mlpp, \
             tc.tile_pool(name="wup", bufs=6) as wup:
            wu_tiles = []
            for ot in range(NFT):
                wt = wup.tile([P, NCT, P], bf16, name="wu", tag="wu")
                nc.sync.dma_start(wt[:], d["WupT"][ot])
                wu_tiles.append(wt)

            x1T = mlpp.tile([P, NCT, 512], f32, name="x1T")
            g2 = mlpp.tile([P, NCT, 512], bf16, name="g2")
            hT = mlpp.tile([P, NFT, 512], bf16, name="hT")

            # ---- proj + residual + LN2 + MLP up, pipelined by token half ----
            # emission order keeps PE fed: proj(0), proj(1), ones(0), ones(1);
            # LN2 stats/normalize for half 0 run on ACT/DVE while PE starts
            # up(0); half 1's LN2 hides under up(0)'s matmuls.
            with tc.tile_pool(name="ln2p", bufs=1) as ln2p, \
                 tc.tile_pool(name="pps", bufs=2, space="PSUM") as pps, \
                 tc.tile_pool(name="lnps", bufs=4, space="PSUM") as lnps, \
                 tc.tile_pool(name="upps", bufs=2, space="PSUM") as upps:
                x1b = ln2p.tile([P, NCT, 512], bf16, name="x1b")
                sqb = ln2p.tile([P, NCT, 512], bf16, name="sqb")
                stats = {}

                def emit_proj(th):
                    sl = slice(th * 256, (th + 1) * 256)
                    for ot in range(NCT):
                        pp = pps.tile([P, 256], f32, name="pp", tag="pp")
                        for dk in range(4):
                            nc.tensor.matmul(
                                pp[:], wpT[:, 2 * dk:2 * dk + 2,
                                           ot * P:(ot + 1) * P],
                                attnT[:, 2 * dk:2 * dk + 2, sl],
                                start=(dk == 0), stop=(dk == 3), perf_mode=DR)
                        ps = ln2p.tile([P, 256], f32, name="ps", tag="ps",
                                       bufs=2)
                        nc.scalar.mul(ps[:], pp[:], IWS)
                        nc.vector.tensor_add(x1T[:, ot, sl], ps[:],
                                             xbT[:, ot, sl])
                        nc.vector.tensor_copy(x1b[:, ot, sl], x1T[:, ot, sl])
                        nc.scalar.activation(sqb[:, ot, sl], x1T[:, ot, sl],
                                             AF.Square)

                def emit_ones(th):
                    sl = slice(th * 256, (th + 1) * 256)
                    psmu = lnps.tile([P, 256], f32, name="psmu", tag="ln")
                    pssq = lnps.tile([P, 256], f32, name="pssq", tag="ln")
                    for ct in range(NCT):
                        nc.tensor.matmul(psmu[:], onesb[:], x1b[:, ct, sl],
                                         start=(ct == 0), stop=(ct == NCT - 1))
                    for ct in range(NCT):
                        nc.tensor.matmul(pssq[:], onesb[:], sqb[:, ct, sl],
                                         start=(ct == 0), stop=(ct == NCT - 1))
                    stats[th] = (psmu, pssq)

                def emit_norm(th):
                    sl = slice(th * 256, (th + 1) * 256)
                    psmu, pssq = stats.pop(th)
                    mu = ln2p.tile([P, 256], f32, name="mu", tag="mu", bufs=2)
                    nc.scalar.mul(mu[:], psmu[:], 1.0 / C)
                    e2 = ln2p.tile([P, 256], f32, name="e2", tag="e2", bufs=2)
                    nc.scalar.mul(e2[:], pssq[:], 1.0 / C)
                    musq = ln2p.tile([P, 256], f32, name="musq", tag="musq",
                                     bufs=2)
                    nc.scalar.activation(musq[:], mu[:], AF.Square)
                    nc.vector.tensor_sub(e2[:], e2[:], musq[:])
                    std = ln2p.tile([P, 256], f32, name="std", tag="std",
                                    bufs=2)
                    nc.scalar.activation(std[:], e2[:], AF.Sqrt, bias=epsT[:])
                    nc.vector.reciprocal(std[:], std[:])
                    for ct in range(NCT):
                        tmpc = ln2p.tile([P, 256], f32, name="tmpc",
                                         tag="tmpc", bufs=2)
                        nc.vector.tensor_sub(tmpc[:], x1T[:, ct, sl], mu[:])
                        nc.vector.tensor_mul(g2[:, ct, sl], tmpc[:], std[:])

                def emit_up():
                    # both halves per f-tile so each wu stream buffer is
                    # fully consumed before its reuse (avoids WAR deadlock)
                    for ot in range(NFT):
                        for th in range(2):
                            sl = slice(th * 256, (th + 1) * 256)
                            pu = upps.tile([P, 256], f32, name="pu", tag="pu")
                            for ct in range(NCT):
                                nc.tensor.matmul(pu[:], wu_tiles[ot][:, ct, :],
                                                 g2[:, ct, sl],
                                                 start=(ct == 0),
                                                 stop=(ct == NCT - 1))
                            nc.scalar.activation(hT[:, ot, sl], pu[:], AF.Gelu,
                                                 bias=bup[:, ot:ot + 1])

                emit_proj(0)
                emit_proj(1)
                emit_ones(0)
                emit_ones(1)
                emit_norm(0)
                emit_norm(1)
                emit_up()

            if DEBUG:
                nc.sync.dma_start(d["dbg_x1"], x1T[:])
                nc.sync.dma_start(d["dbg_g2"], g2[:])

            # ---- MLP down + residual ----
            with tc.tile_pool(name="outp", bufs=1) as outp, \
                 tc.tile_pool(name="dps", bufs=2, space="PSUM") as dps:
                outdst = d["OUT"].rearrange("(ot p) t -> ot p t", p=P)
                for ot in range(NCT):
                    pd = dps.tile([P, 512], f32, name="pd", tag="pd")
                    for cf in range(NFT):
                        nc.tensor.matmul(pd[:], wd[:, cf, ot * P:(ot + 1) * P],
                                         hT[:, cf, :],
                                         start=(cf == 0), stop=(cf == NFT - 1))
                    td = outp.tile([P, 512], f32, name="td", tag="td", bufs=2)
                    nc.scalar.add(td[:], pd[:], bdown[:, ot:ot + 1])
                    ou = outp.tile([P, 512], f32, name="ou", tag="ou", bufs=2)
                    nc.vector.tensor_add(ou[:], td[:], x1T[:, ot, :])
                    nc.gpsimd.dma_start(outdst[ot], ou[:])


def _prep_inputs(x, ln1_w, ln1_b, c_attn_w, c_attn_b, c_proj_w, c_proj_b,
                 ln2_w, ln2_b, up_w, up_b, down_w, down_b):
    """Host-side preprocessing -> list of 8 per-core input dicts."""
    x = np.asarray(x, np.float32)
    f64 = np.float64
    # LN1 on host (pure function of the input)
    mu = x.mean(-1, keepdims=True, dtype=f64)
    var = np.asarray(x, f64).var(-1, keepdims=True)
    g = ((x - mu) / np.sqrt(var + EPS)).astype(np.float32)     # [B, T, C]

    ln1_w = np.asarray(ln1_w, np.float32); ln1_b = np.asarray(ln1_b, np.float32)
    ln2_w = np.asarray(ln2_w, np.float32); ln2_b = np.asarray(ln2_b, np.float32)
    c_attn_w = np.asarray(c_attn_w, np.float32)
    c_attn_b = np.asarray(c_attn_b, np.float32)
    c_proj_w = np.asarray(c_proj_w, np.float32)
    c_proj_b = np.asarray(c_proj_b, np.float32)
    up_w = np.asarray(up_w, np.float32); up_b = np.asarray(up_b, np.float32)
    down_w = np.asarray(down_w, np.float32)
    down_b = np.asarray(down_b, np.float32)

    Wa = c_attn_w * ln1_w[None, :]                  # fold LN1 scale
    ba = c_attn_b + c_attn_w @ ln1_b                # fold LN1 shift
    Wq, Wk, Wv = Wa[:C], Wa[C:2 * C], Wa[2 * C:]
    bqv, bkv, bvv = ba[:C], ba[C:2 * C], ba[2 * C:]
    s = 1.0 / np.sqrt(D)
    Wq = Wq * s; bqv = bqv * s                      # fold attention scale

    Wup = up_w * ln2_w[None, :]
    bupv = up_b + up_w @ ln2_b

    # causal masks for kv tile vs 256-row q subchunk (diagonal tiles), same
    # on every core; tiled x4 across the 4 packed heads
    tk = np.arange(P)[:, None]
    tq = np.arange(SUB)[None, :]
    mA = np.tile((tk <= tq).astype(np.float32), (1, 4))
    mB = np.tile((tk + P <= tq).astype(np.float32), (1, 4))

    shared = {
        "WpT": np.ascontiguousarray(c_proj_w.T * WS).astype(F8),
        "WupT": np.ascontiguousarray(
            Wup.reshape(NFT, P, NCT, P).transpose(0, 3, 2, 1)).astype(BF),
        "bup": np.ascontiguousarray(bupv.reshape(NFT, P).T).astype(np.float32),
        "WdownT": np.ascontiguousarray(
            down_w.T.reshape(NFT, P, C)).astype(BF),
        "bdown": np.ascontiguousarray(
            down_b.reshape(NCT, P).T).astype(np.float32),
        "maskA": mA.astype(BF), "maskB": mB.astype(BF),
    }

    xb = x + c_proj_b[None, None, :]                # fold proj bias in residual
    gT_b = [np.ascontiguousarray(g[b].T).astype(BF) for b in range(B)]
    xbT_b = [np.ascontiguousarray(xb[b].T).astype(np.float32) for b in range(B)]

    # head-pair reorder for q/k: col (hp*128 + (h%2)*64 + d) = head 4j+2hp+(h%2)
    def qk_slice(W, bias, j):
        rows = W.reshape(H, D, C)[4 * j:4 * j + 4]      # [4, 64, C]
        b4 = bias.reshape(H, D)[4 * j:4 * j + 4]
        order = [0, 1, 2, 3]                            # hp0: h0,h1; hp1: h2,h3
        rows = rows[order].reshape(2, 2 * D, C)         # [hp, 128, C]
        b4 = b4[order].reshape(2, 2 * D)
        WT = np.ascontiguousarray(rows.reshape(256, C).T).astype(BF)
        bT = np.ascontiguousarray(b4.reshape(2, P).T).astype(np.float32)
        return WT, bT

    in_maps = []
    for core in range(8):
        b, j = core // 4, core % 4
        m = dict(shared)
        m["gT"] = gT_b[b]
        WqTs, bqs = qk_slice(Wq, bqv, j)
        WkTs, bks = qk_slice(Wk, bkv, j)
        m["WqT"] = WqTs; m["bq"] = bqs
        m["WkT"] = WkTs; m["bk"] = bks
        m["WvT"] = np.ascontiguousarray(
            Wv[256 * j:256 * (j + 1)].T).astype(BF)
        m["brep"] = np.broadcast_to(
            bvv[256 * j:256 * (j + 1)].astype(BF), (P, 256)).copy()
        m["xbT"] = np.concatenate(
            [xbT_b[0][:, 256 * core:256 * (core + 1)],
             xbT_b[1][:, 256 * core:256 * (core + 1)]], axis=1)
        in_maps.append(m)
    return in_maps


def kernel(**inputs):
    global _CACHED_NC
    if _CACHED_NC is None:
        _CACHED_NC = _build_nc()
    nc = _CACHED_NC
    in_maps = _prep_inputs(**inputs)
    try:
        res = run_bass_kernel_spmd(nc, in_maps, list(range(8)))
    except Exception:
        # one retry: transient NRT device faults are recoverable on re-run
        res = run_bass_kernel_spmd(nc, in_maps, list(range(8)))
    out = np.empty((B, T, C), np.float32)
    for core in range(8):
        o = res.results[core]["OUT"]                # [C, 512]
        out[0, 256 * core:256 * (core + 1), :] = o[:, :256].T
        out[1, 256 * core:256 * (core + 1), :] = o[:, 256:].T
    return out
